# revision 19
# baseline (speedup 1.0000x reference)
"""Trainium2 Bass kernel for nn_AutoSelectAttention (parametric Gaussian span scores).

Computes y[b,m,k] = -(((x[k] + mean[b,m]) / (softness[b,m] + EPS))**2) + intercept[b,m]
for x[k] = k - (L-1), k in [0, 2L-1).

Sharding: the fused batch*heads dim (32) is split 4-per-core across 8 NeuronCores;
each core's [4*1024, 2047] output band is independent (no collectives).

Per-core schedule (DMA-write-roofline bound, ~33.5 MB f32 out per core):
  - host precomputes per-token planes [mean, -1/(s+eps)^2, intercept] -> one
    small input DMA; its completion (~9.6us incl. fixed preamble) gates compute.
  - x grid [128, 2047] fp16 (exact for |int| <= 2048) built as one 512-col
    gpsimd iota + three DVE +const shifts so it's ready before the planes
    semaphore (a full-width iota would block the first block until ~11.9us).
  - per 128-token block: ACT Square (z2 = (x+mean)^2, f32) then one DVE
    tensor_scalar (y = z2*ninv2 + intercept) into a grouped output tile.
  - output DRAM is group-contiguous (y1[2,128,W], y2[3,128,2W],
    y4[6,128,4W]): each group is one fully contiguous 1/2/4MB DRAM region
    whose per-partition descriptor is g*8188 contiguous bytes (32KB
    descriptors run at ~430 GB/s, 98% of the per-core write-side wall).
    Ramp single,single,pair,pair,quad x6,pair starts the write stream at
    ~14us and minimizes DMA instruction count (12), which also keeps the
    end-of-kernel semaphore drain short.
  - all DMAs keep the full 128 partitions: the descriptor generator splits
    P partitions over the largest n <= 16 with P % n == 0, and any config
    mixing in 112/120/124-partition DMAs measured 25-60% slower chip-wide
    under 8-core HBM contention (tested and rejected).
"""

import sys

import numpy as np

for _p in ("/opt/trn_rl_repo", "/root/.axon_site", "/opt/pypackages"):
    if _p not in sys.path:
        sys.path.append(_p)

L = 1024
W = 2 * L - 1  # 2047
BH = 32
M = 1024
EPS = 1e-5
NCORES = 8
BH_SH = BH // NCORES  # 4
ROWS = BH_SH * M  # 4096 tokens per core
P = 128
NBLK = ROWS // P  # 32 blocks of 128 tokens

# Output DMA grouping ramp (must sum to NBLK): small groups early so the DMA
# starts streaming ASAP, quads in steady state for 32KB contiguous descriptors.
GROUPS = [1, 1, 2, 2, 4, 4, 4, 4, 4, 4, 2]
assert sum(GROUPS) == NBLK

_NC_CACHE = {}


def _build_nc():
    import concourse.bacc as bacc
    import concourse.tile as tile
    from concourse import mybir

    f32 = mybir.dt.float32
    f16 = mybir.dt.float16
    Sq = mybir.ActivationFunctionType.Square

    nc = bacc.Bacc("TRN2", target_bir_lowering=False, debug=False)
    # planes[p, 0, k] = mean, [p, 1, k] = -1/(softness+EPS)^2, [p, 2, k] =
    # intercept for token t = k*128 + p (host-precomputed).
    planes = nc.dram_tensor("planes", [P, 3, NBLK], f32, kind="ExternalInput").ap()
    # One output tensor per group size; group i of size g occupies one fully
    # contiguous g*1MB DRAM region laid out [partition, g*W] so every
    # partition's descriptor is g*8188 contiguous bytes and partitions are
    # adjacent (y*[i, p, j*W+w] = out[token (k0+j)*128+p, w]).
    n_by_g = {g: GROUPS.count(g) for g in set(GROUPS)}
    youts = {
        g: nc.dram_tensor(f"y{g}", [n, P, g * W], f32, kind="ExternalOutput").ap()
        for g, n in sorted(n_by_g.items())
    }

    with tile.TileContext(nc) as tc:
        with (
            tc.tile_pool(name="const", bufs=1) as cpool,
            tc.tile_pool(name="work", bufs=3) as wpool,
            tc.tile_pool(name="o1", bufs=2) as o1pool,
            tc.tile_pool(name="o2", bufs=3) as o2pool,
            tc.tile_pool(name="o4", bufs=2) as o4pool,
        ):
            # Warmup ACTIVATE with no data dependencies: pulls the ~1.5us
            # Square table load to kernel start instead of serializing it
            # behind the planes DMA.
            warm = cpool.tile([P, 1], f32)
            one = nc.const_aps.tensor(1.0, (P, 1))
            nc.scalar.activation(warm[:], one, Sq, bias=0.0, scale=1.0)

            # x grid in fp16 (integers |x| <= 2047 are exact in fp16).
            xb = cpool.tile([P, 2 * L], f16)
            nc.gpsimd.iota(
                xb[:, 0:512],
                [[1, 512]],
                base=-(L - 1),
                channel_multiplier=0,
                allow_small_or_imprecise_dtypes=True,
            )
            for j in (1, 2, 3):
                nc.vector.tensor_scalar(
                    xb[:, j * 512 : (j + 1) * 512],
                    xb[:, 0:512],
                    float(j * 512),
                    None,
                    mybir.AluOpType.add,
                )

            spn = cpool.tile([P, 3, NBLK], f32)
            nc.sync.dma_start(spn[:], planes[:, :, :])

            pools = {1: o1pool, 2: o2pool, 4: o4pool}
            gidx = {g: 0 for g in n_by_g}
            k = 0
            for g in GROUPS:
                ot = pools[g].tile([P, g * W], f32)
                for j in range(g):
                    kk = k + j
                    # z2 = (x + mean)^2 on ACT (per-partition bias = mean)
                    z2 = wpool.tile([P, W], f32)
                    nc.scalar.activation(
                        z2[:], xb[:, 0:W], Sq, bias=spn[:, 0, kk : kk + 1], scale=1.0
                    )
                    # y = z2 * ninv2 + intercept on DVE (per-partition scalars)
                    nc.vector.tensor_scalar(
                        ot[:, j * W : (j + 1) * W],
                        z2[:],
                        spn[:, 1, kk : kk + 1],
                        spn[:, 2, kk : kk + 1],
                        mybir.AluOpType.mult,
                        mybir.AluOpType.add,
                    )
                i = gidx[g]
                nc.sync.dma_start(youts[g][i : i + 1, :, :], ot[:])
                gidx[g] += 1
                k += g
    nc.compile()
    return nc


def _get_nc():
    if "nc" not in _NC_CACHE:
        _NC_CACHE["nc"] = _build_nc()
    return _NC_CACHE["nc"]


def _make_in_maps(span: np.ndarray) -> list[dict]:
    span = np.ascontiguousarray(span, dtype=np.float32)
    in_maps = []
    for c in range(NCORES):
        # [blk, p, comp] with token t = blk*128 + p
        shard = span[c * BH_SH : (c + 1) * BH_SH].reshape(NBLK, P, 3)
        mean = shard[:, :, 0].T  # [p, blk]
        soft = shard[:, :, 1].T.astype(np.float64)
        cept = shard[:, :, 2].T
        ninv2 = (-1.0 / (soft + EPS) ** 2).astype(np.float32)
        planes = np.ascontiguousarray(
            np.stack([mean, ninv2, cept], axis=1), dtype=np.float32
        )  # [128, 3, NBLK]
        in_maps.append({"planes": planes})
    return in_maps


def kernel(span: np.ndarray, _trace: bool = False, _tmpdir: str | None = None):
    from concourse.bass_utils import run_bass_kernel_spmd

    nc = _get_nc()
    in_maps = _make_in_maps(span)
    res = run_bass_kernel_spmd(
        nc,
        in_maps,
        core_ids=list(range(NCORES)),
        trace=_trace,
        tmpdir=_tmpdir,
    )
    # Reassemble each core's [ROWS, W] band from the group-contiguous
    # tensors: group i of size g holds [P, g, W] with token t = (k0+j)*128+p.
    shards = []
    for r in res.results:
        band = np.empty((ROWS, W), np.float32)
        gidx = {g: 0 for g in set(GROUPS)}
        k = 0
        for g in GROUPS:
            i = gidx[g]
            arr = np.asarray(r[f"y{g}"]).reshape(-1, P, g * W)[i]
            band[k * P : (k + g) * P, :] = (
                arr.reshape(P, g, W).transpose(1, 0, 2).reshape(g * P, W)
            )
            gidx[g] += 1
            k += g
        shards.append(band.reshape(BH_SH, M, W))
    out = np.concatenate(shards, axis=0).astype(np.float32)
    if _trace:
        kernel.last_results = res
    return out


# revision 20
# speedup vs baseline: 1.0352x; 1.0352x over previous
"""Trainium2 Bass kernel for nn_AutoSelectAttention (parametric Gaussian span scores).

Computes y[b,m,k] = -(((x[k] + mean[b,m]) / (softness[b,m] + EPS))**2) + intercept[b,m]
for x[k] = k - (L-1), k in [0, 2L-1).

Sharding: the fused batch*heads dim (32) is split 4-per-core across 8 NeuronCores;
each core's [4*1024, 2047] output band is independent (no collectives).

Per-core schedule (DMA-write-roofline bound, ~33.5 MB f32 out per core):
  - host precomputes per-token planes [mean, -1/(s+eps)^2, intercept] -> one
    small input DMA; its completion (~9.6us incl. fixed preamble) gates compute.
  - x grid fp16 (exact for |int| <= 2048) built as one 512-col gpsimd iota +
    three DVE +const shifts so it's ready before the planes semaphore.
  - per block: ACT Square (z2 = (x+mean)^2, f32) then one DVE tensor_scalar
    (y = z2*ninv2 + intercept) into a grouped output tile.
  - blocks are 120 tokens tall: a DMA with P partitions is split evenly over
    the largest engine count n <= 16 with P % n == 0 (software assignment --
    124 partitions went to only 4 engines x 31), so P=120 -> 15 engines x 8
    and SDMA engine 15 is never used.  Engine 15 is stochastically 20-50%
    slower (shared-port contention; sick on most recent runs, +14-17us when
    it carries a 1/16 share).  All DMAs keep the SAME [120, g*W] shape:
    configs MIXING 128-partition and 112/120-partition DMAs measured 25-60%
    slower chip-wide and were rejected; uniform shape avoids that mode.
  - 34 blocks cover tokens 0..4079; the last 16 tokens per core (0.4%) are
    computed on host.
  - output DRAM is group-contiguous (y1[2,120,W] singles, y2[16,120,2W]
    pairs): each group is one contiguous DRAM region whose per-partition
    descriptor is g*8188 contiguous bytes (16KB descriptors run ~426 GB/s,
    within 1% of the write-side wall).  Two singles start the write stream
    at ~14us; all-pairs steady state keeps DMA readiness smooth at any
    compute cadence, so the stream runs gap-free.
"""

import sys

import numpy as np

for _p in ("/opt/trn_rl_repo", "/root/.axon_site", "/opt/pypackages"):
    if _p not in sys.path:
        sys.path.append(_p)

L = 1024
W = 2 * L - 1  # 2047
BH = 32
M = 1024
EPS = 1e-5
NCORES = 8
BH_SH = BH // NCORES  # 4
ROWS = BH_SH * M  # 4096 tokens per core
P = 120  # tokens per block; 15 SDMA engines x 8 partitions, engine 15 idle
NBLK = 34  # full blocks on hardware
HW_ROWS = NBLK * P  # 4080; tokens 4080..4095 are computed on host
GROUPS = [1, 1] + [2] * 16
assert sum(GROUPS) == NBLK

_NC_CACHE = {}


def _build_nc():
    import concourse.bacc as bacc
    import concourse.tile as tile
    from concourse import mybir

    f32 = mybir.dt.float32
    f16 = mybir.dt.float16
    Sq = mybir.ActivationFunctionType.Square

    nc = bacc.Bacc("TRN2", target_bir_lowering=False, debug=False)
    # planes[p, 0, k] = mean, [p, 1, k] = -1/(softness+EPS)^2, [p, 2, k] =
    # intercept for token t = k*120 + p (host-precomputed).
    planes = nc.dram_tensor("planes", [P, 3, NBLK], f32, kind="ExternalInput").ap()
    # One output tensor per group size; group i of size g occupies one fully
    # contiguous DRAM region laid out [partition, g*W] so every partition's
    # descriptor is g*8188 contiguous bytes and partitions are adjacent
    # (y*[i, p, j*W+w] = out[token (k0+j)*120+p, w]).
    n_by_g = {g: GROUPS.count(g) for g in set(GROUPS)}
    youts = {
        g: nc.dram_tensor(f"y{g}", [n, P, g * W], f32, kind="ExternalOutput").ap()
        for g, n in sorted(n_by_g.items())
    }

    with tile.TileContext(nc) as tc:
        with (
            tc.tile_pool(name="const", bufs=1) as cpool,
            tc.tile_pool(name="work", bufs=3) as wpool,
            tc.tile_pool(name="o1", bufs=2) as o1pool,
            tc.tile_pool(name="o2", bufs=5) as o2pool,
        ):
            # Warmup ACTIVATE with no data dependencies: pulls the ~1.5us
            # Square table load to kernel start instead of serializing it
            # behind the planes DMA.
            warm = cpool.tile([P, 1], f32)
            one = nc.const_aps.tensor(1.0, (P, 1))
            nc.scalar.activation(warm[:], one, Sq, bias=0.0, scale=1.0)

            # x grid in fp16 (integers |x| <= 2047 are exact in fp16).
            xb = cpool.tile([P, 2 * L], f16)
            nc.gpsimd.iota(
                xb[:, 0:512],
                [[1, 512]],
                base=-(L - 1),
                channel_multiplier=0,
                allow_small_or_imprecise_dtypes=True,
            )
            for j in (1, 2, 3):
                nc.vector.tensor_scalar(
                    xb[:, j * 512 : (j + 1) * 512],
                    xb[:, 0:512],
                    float(j * 512),
                    None,
                    mybir.AluOpType.add,
                )

            spn = cpool.tile([P, 3, NBLK], f32)
            nc.sync.dma_start(spn[:], planes[:, :, :])

            pools = {1: o1pool, 2: o2pool}
            gidx = {g: 0 for g in n_by_g}
            k = 0
            for g in GROUPS:
                ot = pools[g].tile([P, g * W], f32)
                for j in range(g):
                    kk = k + j
                    # z2 = (x + mean)^2 on ACT (per-partition bias = mean)
                    z2 = wpool.tile([P, W], f32)
                    nc.scalar.activation(
                        z2[:], xb[:, 0:W], Sq, bias=spn[:, 0, kk : kk + 1], scale=1.0
                    )
                    # y = z2 * ninv2 + intercept on DVE (per-partition scalars)
                    nc.vector.tensor_scalar(
                        ot[:, j * W : (j + 1) * W],
                        z2[:],
                        spn[:, 1, kk : kk + 1],
                        spn[:, 2, kk : kk + 1],
                        mybir.AluOpType.mult,
                        mybir.AluOpType.add,
                    )
                i = gidx[g]
                nc.sync.dma_start(youts[g][i : i + 1, :, :], ot[:])
                gidx[g] += 1
                k += g
    nc.compile()
    return nc


def _get_nc():
    if "nc" not in _NC_CACHE:
        _NC_CACHE["nc"] = _build_nc()
    return _NC_CACHE["nc"]


def _make_in_maps(span: np.ndarray) -> list[dict]:
    span = np.ascontiguousarray(span, dtype=np.float32)
    in_maps = []
    for c in range(NCORES):
        flat = span[c * BH_SH : (c + 1) * BH_SH].reshape(ROWS, 3)
        # [blk, p, comp] with token t = blk*120 + p (hardware tokens only)
        shard = flat[:HW_ROWS].reshape(NBLK, P, 3)
        mean = shard[:, :, 0].T  # [p, blk]
        soft = shard[:, :, 1].T.astype(np.float64)
        cept = shard[:, :, 2].T
        ninv2 = (-1.0 / (soft + EPS) ** 2).astype(np.float32)
        planes = np.ascontiguousarray(
            np.stack([mean, ninv2, cept], axis=1), dtype=np.float32
        )  # [120, 3, NBLK]
        in_maps.append({"planes": planes})
    return in_maps


def kernel(span: np.ndarray, _trace: bool = False, _tmpdir: str | None = None):
    from concourse.bass_utils import run_bass_kernel_spmd

    span = np.ascontiguousarray(span, dtype=np.float32)
    nc = _get_nc()
    in_maps = _make_in_maps(span)
    res = run_bass_kernel_spmd(
        nc,
        in_maps,
        core_ids=list(range(NCORES)),
        trace=_trace,
        tmpdir=_tmpdir,
    )
    # Host-side values for the 16-token remainder (tokens HW_ROWS..ROWS-1).
    x = (np.arange(W, dtype=np.float32) - np.float32(L - 1))[None, :]

    # Reassemble each core's [ROWS, W] band from the group-contiguous
    # tensors: group i of size g holds [P, g, W] with token t = (k0+j)*120+p.
    shards = []
    for c, r in enumerate(res.results):
        band = np.empty((ROWS, W), np.float32)
        gidx = {g: 0 for g in set(GROUPS)}
        k = 0
        for g in GROUPS:
            i = gidx[g]
            arr = np.asarray(r[f"y{g}"]).reshape(-1, P, g * W)[i]
            band[k * P : (k + g) * P, :] = (
                arr.reshape(P, g, W).transpose(1, 0, 2).reshape(g * P, W)
            )
            gidx[g] += 1
            k += g
        rem = span[c * BH_SH : (c + 1) * BH_SH].reshape(ROWS, 3)[HW_ROWS:]
        band[HW_ROWS:] = (
            -(((x + rem[:, 0:1]) / (rem[:, 1:2] + np.float32(EPS))) ** 2)
            + rem[:, 2:3]
        ).astype(np.float32)
        shards.append(band.reshape(BH_SH, M, W))
    out = np.concatenate(shards, axis=0).astype(np.float32)
    if _trace:
        kernel.last_results = res
    return out
